# revision 2
# baseline (speedup 1.0000x reference)
"""Trainium2 Bass kernel for per-pixel dot-product attention.

Reference op (per pixel, over C=80 channels split q/k/v = 8/64/8):
    qk[v] = sum_k q[k] * K[k, v] / sqrt(8)
    attn  = softmax(qk over v)
    out[v] = attn[v] * V[v]

Strategy: pure data-parallel over 8 NeuronCores — core i handles batch
i//2, H-rows half (i%2).  Per core all compute is elementwise on
(128, ncol) pixel grids; the 80 channels live as column-blocks of big
SBUF tiles so the whole per-pixel matvec+softmax is ~11 wide vector ops
per chunk (no PSUM / TensorE / transposes).  DVE does the multiplies &
small adds, GPSIMD the big add-tree levels, ScalarE the exp.
"""

import numpy as np

NK = 8
NV = 8
C = NK + NK * NV + NV  # 80
B, H, W = 4, 512, 512
N_CORES = 8
ROWS = H // 2            # rows per core
PIX = ROWS * W           # pixels per core (131072)
NCHUNK = 8               # chunks per core
_SCALE = 1.0 / float(np.sqrt(NK))


def _ensure_path():
    import sys
    p = "/opt/trn_rl_repo"
    if p not in sys.path:
        sys.path.insert(0, p)


def build_nc(pix=PIX, nchunk=NCHUNK, pool_adds=True, recip_on_act=False):
    """Build the per-core Bass program for a (80, pix) f32 shard."""
    _ensure_path()
    import concourse.tile as tile
    from concourse import bacc, mybir

    f32 = mybir.dt.float32
    npix = pix // nchunk
    assert npix % 128 == 0
    ncol = npix // 128

    nc = bacc.Bacc("TRN2", target_bir_lowering=False, debug=False)
    x = nc.dram_tensor("x", [C, pix], f32, kind="ExternalInput")
    y = nc.dram_tensor("y", [NV, pix], f32, kind="ExternalOutput")

    with tile.TileContext(nc) as tc:
        with (
            tc.tile_pool(name="inp", bufs=2) as in_pool,
            tc.tile_pool(name="work", bufs=1) as work_pool,
            tc.tile_pool(name="pipe", bufs=2) as pipe_pool,
        ):
            for j in range(nchunk):
                off = j * npix
                q_t = in_pool.tile([128, NK * ncol], f32, name=f"q{j}", tag="q")
                k_t = in_pool.tile([128, NK * NV * ncol], f32, name=f"k{j}", tag="k")
                v_t = in_pool.tile([128, NV * ncol], f32, name=f"v{j}", tag="v")

                nc.sync.dma_start(
                    out=q_t.rearrange("p (c x) -> p c x", c=NK),
                    in_=x[0:NK, off:off + npix].rearrange("c (p x) -> p c x", p=128),
                )
                # k_t column-block b = v*NK + k holds input channel NK + k*NV + v
                k4 = k_t.rearrange("p (v k x) -> p v k x", v=NV, k=NK)
                for kk in range(NK):
                    nc.sync.dma_start(
                        out=k4[:, :, kk:kk + 1, :].squeeze(2),
                        in_=x[NK + kk * NV:NK + (kk + 1) * NV, off:off + npix]
                        .rearrange("c (p x) -> p c x", p=128),
                    )
                nc.sync.dma_start(
                    out=v_t.rearrange("p (c x) -> p c x", c=NV),
                    in_=x[NK + NK * NV:C, off:off + npix]
                    .rearrange("c (p x) -> p c x", p=128),
                )

                # prod[v,k] = q[k] * K[k,v]   (one broadcast multiply)
                prod = work_pool.tile([128, NK * NV * ncol], f32, name=f"prod{j}", tag="prod")
                p4 = prod.rearrange("p (v k x) -> p v k x", v=NV, k=NK)
                q_b = (
                    q_t.rearrange("p (k x) -> p k x", k=NK)
                    .unsqueeze(1)
                    .broadcast_to((128, NV, NK, ncol))
                )
                nc.vector.tensor_tensor(p4, q_b, k4, mybir.AluOpType.mult)

                # sum over k: 3-level pairwise tree
                eng = nc.gpsimd if pool_adds else nc.vector
                l1 = work_pool.tile([128, NV * 4 * ncol], f32, name=f"l1_{j}", tag="l1")
                l14 = l1.rearrange("p (v k x) -> p v k x", v=NV, k=4)
                eng.tensor_tensor(l14, p4[:, :, 0:4, :], p4[:, :, 4:8, :], mybir.AluOpType.add)
                l2 = work_pool.tile([128, NV * 2 * ncol], f32, name=f"l2_{j}", tag="l2")
                l24 = l2.rearrange("p (v k x) -> p v k x", v=NV, k=2)
                eng.tensor_tensor(l24, l14[:, :, 0:2, :], l14[:, :, 2:4, :], mybir.AluOpType.add)
                qk = pipe_pool.tile([128, NV * ncol], f32, name=f"qk{j}", tag="qk")
                qk4 = qk.rearrange("p (v x) -> p v x", v=NV).unsqueeze(2)
                nc.vector.tensor_tensor(qk4, l24[:, :, 0:1, :], l24[:, :, 1:2, :], mybir.AluOpType.add)

                # e = exp(qk / sqrt(NK)); softmax denominators over v
                e = pipe_pool.tile([128, NV * ncol], f32, name=f"e{j}", tag="e")
                nc.scalar.activation(e, qk, mybir.ActivationFunctionType.Exp, scale=_SCALE)
                t1 = pipe_pool.tile([128, 4 * ncol], f32, name=f"t1_{j}", tag="t1")
                nc.vector.tensor_tensor(t1, e[:, 0:4 * ncol], e[:, 4 * ncol:], mybir.AluOpType.add)
                t2 = pipe_pool.tile([128, 2 * ncol], f32, name=f"t2_{j}", tag="t2")
                nc.vector.tensor_tensor(t2, t1[:, 0:2 * ncol], t1[:, 2 * ncol:], mybir.AluOpType.add)
                s = pipe_pool.tile([128, ncol], f32, name=f"s{j}", tag="s")
                nc.vector.tensor_tensor(s, t2[:, 0:ncol], t2[:, ncol:], mybir.AluOpType.add)
                r = pipe_pool.tile([128, ncol], f32, name=f"r{j}", tag="r")
                if recip_on_act:
                    ls = pipe_pool.tile([128, ncol], f32, name=f"ls{j}", tag="ls")
                    nc.scalar.activation(ls, s, mybir.ActivationFunctionType.Log)
                    nc.scalar.activation(r, ls, mybir.ActivationFunctionType.Exp, scale=-1.0)
                else:
                    nc.vector.reciprocal(r, s)

                # out[v] = e[v] * V[v] * r
                m1 = pipe_pool.tile([128, NV * ncol], f32, name=f"m1_{j}", tag="m1")
                nc.vector.tensor_tensor(m1, e, v_t, mybir.AluOpType.mult)
                o = pipe_pool.tile([128, NV * ncol], f32, name=f"o{j}", tag="o")
                r_b = r.unsqueeze(1).broadcast_to((128, NV, ncol))
                nc.vector.tensor_tensor(
                    o.rearrange("p (v x) -> p v x", v=NV),
                    m1.rearrange("p (v x) -> p v x", v=NV),
                    r_b,
                    mybir.AluOpType.mult,
                )
                for v in range(NV):
                    nc.sync.dma_start(
                        out=y[v, off:off + npix].rearrange("(p x) -> p x", p=128),
                        in_=o[:, v * ncol:(v + 1) * ncol],
                    )
    nc.compile()
    return nc


_NC_CACHE = {}


def _get_nc():
    key = (PIX, NCHUNK)
    if key not in _NC_CACHE:
        _NC_CACHE[key] = build_nc()
    return _NC_CACHE[key]


def make_in_maps(inp):
    in_maps = []
    for core in range(N_CORES):
        b, half = core // 2, core % 2
        shard = np.ascontiguousarray(
            inp[b, :, half * ROWS:(half + 1) * ROWS, :], dtype=np.float32
        ).reshape(C, PIX)
        in_maps.append({"x": shard})
    return in_maps


def assemble_out(results):
    out = np.empty((B, NV, H, W), np.float32)
    for core in range(N_CORES):
        b, half = core // 2, core % 2
        out[b, :, half * ROWS:(half + 1) * ROWS, :] = (
            results[core]["y"].reshape(NV, ROWS, W)
        )
    return out


def run_spmd(inp, trace=False, **kwargs):
    """Run the SPMD kernel on 8 cores; returns (full_output, BassKernelResults)."""
    _ensure_path()
    from concourse.bass_utils import run_bass_kernel_spmd

    inp = np.asarray(inp)
    assert inp.shape == (B, C, H, W), inp.shape
    nc = _get_nc()
    res = run_bass_kernel_spmd(
        nc, make_in_maps(inp), list(range(N_CORES)), trace=trace, **kwargs
    )
    return assemble_out(res.results), res


def kernel(inp):
    out, _ = run_spmd(inp, trace=False)
    return out


# revision 6
# speedup vs baseline: 1.4405x; 1.4405x over previous
"""Trainium2 Bass kernel for per-pixel dot-product attention.

Reference op (per pixel, over C=80 channels split q/k/v = 8/64/8):
    qk[v] = sum_k q[k] * K[k, v] / sqrt(8)
    attn  = softmax(qk over v)
    out[v] = attn[v] * V[v]

Strategy: pure data-parallel over 8 NeuronCores — core i handles batch
i//2, H-rows half (i%2).  Per core all compute is elementwise on
(128, ncol) pixel grids; the 80 channels live as column-blocks of big
SBUF tiles so the whole per-pixel matvec+softmax is ~11 wide vector ops
per chunk (no PSUM / TensorE / transposes).  DVE does the multiplies &
small adds, GPSIMD the big add-tree levels, ScalarE the exp.
"""

import numpy as np

NK = 8
NV = 8
C = NK + NK * NV + NV  # 80
B, H, W = 4, 512, 512
N_CORES = 8
ROWS = H // 2            # rows per core
PIX = ROWS * W           # pixels per core (131072)
NCHUNK = 8               # chunks per core
_SCALE = 1.0 / float(np.sqrt(NK))


def _ensure_path():
    import sys
    p = "/opt/trn_rl_repo"
    if p not in sys.path:
        sys.path.insert(0, p)


def build_nc(pix=PIX, nchunk=NCHUNK, recip_on_act=True):
    """Build the per-core Bass program for a (80, pix) f32 shard.

    All tensor_tensor work runs on DVE (GPSIMD shares an SBUF port with DVE
    and the two engines serialize, so Pool offload is a net loss).  Inputs
    stream in on the sync HWDGE ring, outputs on the scalar ring.
    """
    _ensure_path()
    import concourse.tile as tile
    from concourse import bacc, mybir

    f32 = mybir.dt.float32
    npix = pix // nchunk
    assert npix % 128 == 0
    ncol = npix // 128

    nc = bacc.Bacc("TRN2", target_bir_lowering=False, debug=False)
    x = nc.dram_tensor("x", [C, pix], f32, kind="ExternalInput")
    y = nc.dram_tensor("y", [NV, pix], f32, kind="ExternalOutput")

    with tile.TileContext(nc) as tc:
        with (
            tc.tile_pool(name="inp", bufs=2) as in_pool,
            tc.tile_pool(name="work", bufs=1) as work_pool,
            tc.tile_pool(name="pipe", bufs=2) as pipe_pool,
        ):
            for j in range(nchunk):
                off = j * npix
                q_t = in_pool.tile([128, NK * ncol], f32, name=f"q{j}", tag="q")
                k_t = in_pool.tile([128, NK * NV * ncol], f32, name=f"k{j}", tag="k")
                v_t = in_pool.tile([128, NV * ncol], f32, name=f"v{j}", tag="v")

                nc.sync.dma_start(
                    out=q_t.rearrange("p (c x) -> p c x", c=NK),
                    in_=x[0:NK, off:off + npix].rearrange("c (p x) -> p c x", p=128),
                )
                # k_t column-block b = k*NV + v holds input channel NK + k*NV + v
                # (k-major matches DRAM channel order, so one 3D-balanced DMA)
                k4 = k_t.rearrange("p (k v x) -> p k v x", k=NK, v=NV)
                nc.sync.dma_start(
                    out=k_t.rearrange("p (c x) -> p c x", c=NK * NV),
                    in_=x[NK:NK + NK * NV, off:off + npix]
                    .rearrange("c (p x) -> p c x", p=128),
                )
                nc.sync.dma_start(
                    out=v_t.rearrange("p (c x) -> p c x", c=NV),
                    in_=x[NK + NK * NV:C, off:off + npix]
                    .rearrange("c (p x) -> p c x", p=128),
                )

                # prod[k,v] = q[k] * K[k,v]   (one broadcast multiply)
                prod = work_pool.tile([128, NK * NV * ncol], f32, name=f"prod{j}", tag="prod")
                p4 = prod.rearrange("p (k v x) -> p k v x", k=NK, v=NV)
                q_b = (
                    q_t.rearrange("p (k x) -> p k x", k=NK)
                    .unsqueeze(2)
                    .broadcast_to((128, NK, NV, ncol))
                )
                nc.vector.tensor_tensor(p4, q_b, k4, mybir.AluOpType.mult)

                # sum over k (outer block index): 3-level pairwise tree (all DVE)
                l1 = work_pool.tile([128, 4 * NV * ncol], f32, name=f"l1_{j}", tag="l1")
                l14 = l1.rearrange("p (k v x) -> p k v x", k=4, v=NV)
                nc.vector.tensor_tensor(l14, p4[:, 0:4], p4[:, 4:8], mybir.AluOpType.add)
                l2 = work_pool.tile([128, 2 * NV * ncol], f32, name=f"l2_{j}", tag="l2")
                l24 = l2.rearrange("p (k v x) -> p k v x", k=2, v=NV)
                nc.vector.tensor_tensor(l24, l14[:, 0:2], l14[:, 2:4], mybir.AluOpType.add)
                qk = pipe_pool.tile([128, NV * ncol], f32, name=f"qk{j}", tag="qk")
                qk4 = qk.rearrange("p (v x) -> p v x", v=NV).unsqueeze(1)
                nc.vector.tensor_tensor(qk4, l24[:, 0:1], l24[:, 1:2], mybir.AluOpType.add)

                # e = exp(qk / sqrt(NK)); softmax denominators over v
                e = pipe_pool.tile([128, NV * ncol], f32, name=f"e{j}", tag="e")
                nc.scalar.activation(e, qk, mybir.ActivationFunctionType.Exp, scale=_SCALE)
                t1 = pipe_pool.tile([128, 4 * ncol], f32, name=f"t1_{j}", tag="t1")
                nc.vector.tensor_tensor(t1, e[:, 0:4 * ncol], e[:, 4 * ncol:], mybir.AluOpType.add)
                t2 = pipe_pool.tile([128, 2 * ncol], f32, name=f"t2_{j}", tag="t2")
                nc.vector.tensor_tensor(t2, t1[:, 0:2 * ncol], t1[:, 2 * ncol:], mybir.AluOpType.add)
                s = pipe_pool.tile([128, ncol], f32, name=f"s{j}", tag="s")
                nc.vector.tensor_tensor(s, t2[:, 0:ncol], t2[:, ncol:], mybir.AluOpType.add)
                r = pipe_pool.tile([128, ncol], f32, name=f"r{j}", tag="r")
                if recip_on_act:
                    # r = exp(-ln s): Log and Exp share one ACT table set
                    ls = pipe_pool.tile([128, ncol], f32, name=f"ls{j}", tag="ls")
                    nc.scalar.activation(ls, s, mybir.ActivationFunctionType.Ln)
                    nc.scalar.activation(r, ls, mybir.ActivationFunctionType.Exp, scale=-1.0)
                else:
                    nc.vector.reciprocal(r, s)

                # out[v] = e[v] * V[v] * r
                m1 = pipe_pool.tile([128, NV * ncol], f32, name=f"m1_{j}", tag="m1")
                nc.vector.tensor_tensor(m1, e, v_t, mybir.AluOpType.mult)
                o = pipe_pool.tile([128, NV * ncol], f32, name=f"o{j}", tag="o")
                r_b = r.unsqueeze(1).broadcast_to((128, NV, ncol))
                nc.vector.tensor_tensor(
                    o.rearrange("p (v x) -> p v x", v=NV),
                    m1.rearrange("p (v x) -> p v x", v=NV),
                    r_b,
                    mybir.AluOpType.mult,
                )
                # one output DMA per chunk on the scalar HWDGE ring
                nc.scalar.dma_start(
                    out=y[0:NV, off:off + npix].rearrange("c (p x) -> p c x", p=128),
                    in_=o.rearrange("p (c x) -> p c x", c=NV),
                )
    nc.compile()
    return nc


_NC_CACHE = {}


def _get_nc():
    key = (PIX, NCHUNK)
    if key not in _NC_CACHE:
        _NC_CACHE[key] = build_nc()
    return _NC_CACHE[key]


def make_in_maps(inp):
    in_maps = []
    for core in range(N_CORES):
        b, half = core // 2, core % 2
        shard = np.ascontiguousarray(
            inp[b, :, half * ROWS:(half + 1) * ROWS, :], dtype=np.float32
        ).reshape(C, PIX)
        in_maps.append({"x": shard})
    return in_maps


def assemble_out(results):
    out = np.empty((B, NV, H, W), np.float32)
    for core in range(N_CORES):
        b, half = core // 2, core % 2
        out[b, :, half * ROWS:(half + 1) * ROWS, :] = (
            results[core]["y"].reshape(NV, ROWS, W)
        )
    return out


def run_spmd(inp, trace=False, **kwargs):
    """Run the SPMD kernel on 8 cores; returns (full_output, BassKernelResults)."""
    _ensure_path()
    from concourse.bass_utils import run_bass_kernel_spmd

    inp = np.asarray(inp)
    assert inp.shape == (B, C, H, W), inp.shape
    nc = _get_nc()
    res = run_bass_kernel_spmd(
        nc, make_in_maps(inp), list(range(N_CORES)), trace=trace, **kwargs
    )
    return assemble_out(res.results), res


def kernel(inp):
    out, _ = run_spmd(inp, trace=False)
    return out


# revision 9
# speedup vs baseline: 1.5145x; 1.0514x over previous
"""Trainium2 Bass kernel for per-pixel dot-product attention.

Reference op (per pixel, over C=80 channels split q/k/v = 8/64/8):
    qk[v] = sum_k q[k] * K[k, v] / sqrt(8)
    attn  = softmax(qk over v)
    out[v] = attn[v] * V[v]

Strategy: pure data-parallel over 8 NeuronCores — core i handles batch
i//2, H-rows half (i%2).  Per core all compute is elementwise on
(128, ncol) pixel grids; the 80 channels live as column-blocks of big
SBUF tiles so the whole per-pixel matvec+softmax is ~11 wide vector ops
per chunk (no PSUM / TensorE / transposes).  DVE does the multiplies &
small adds, GPSIMD the big add-tree levels, ScalarE the exp.
"""

import numpy as np

NK = 8
NV = 8
C = NK + NK * NV + NV  # 80
B, H, W = 4, 512, 512
N_CORES = 8
ROWS = H // 2            # rows per core
PIX = ROWS * W           # pixels per core (131072)
NCHUNK = 8               # chunks per core
_SCALE = 1.0 / float(np.sqrt(NK))


def _ensure_path():
    import sys
    p = "/opt/trn_rl_repo"
    if p not in sys.path:
        sys.path.insert(0, p)


def build_nc(pix=PIX, nchunk=NCHUNK, recip_on_act=False, bf16_tree=False):
    """Build the per-core Bass program for a (80, pix) f32 shard.

    All tensor_tensor work runs on DVE (GPSIMD shares an SBUF port with DVE
    and the two engines serialize, so Pool offload is a net loss).  Inputs
    stream in on the sync HWDGE ring, output + V on the scalar ring.  The K
    load and the product are split into k-halves so compute starts after
    half the K data has landed.
    """
    _ensure_path()
    import concourse.tile as tile
    from concourse import bacc, mybir

    f32 = mybir.dt.float32
    mid = mybir.dt.bfloat16 if bf16_tree else f32
    npix = pix // nchunk
    assert npix % 128 == 0
    ncol = npix // 128

    nc = bacc.Bacc("TRN2", target_bir_lowering=False, debug=False)
    x = nc.dram_tensor("x", [C, pix], f32, kind="ExternalInput")
    y = nc.dram_tensor("y", [NV, pix], f32, kind="ExternalOutput")

    with tile.TileContext(nc) as tc:
        with (
            tc.tile_pool(name="inp", bufs=2) as in_pool,
            tc.tile_pool(name="work", bufs=1) as work_pool,
            tc.tile_pool(name="pipe", bufs=2) as pipe_pool,
        ):
            for j in range(nchunk):
                off = j * npix
                q_t = in_pool.tile([128, NK * ncol], f32, name=f"q{j}", tag="q")
                k_t = in_pool.tile([128, NK * NV * ncol], f32, name=f"k{j}", tag="k")
                v_t = in_pool.tile([128, NV * ncol], f32, name=f"v{j}", tag="v")

                nc.sync.dma_start(
                    out=q_t.rearrange("p (c x) -> p c x", c=NK),
                    in_=x[0:NK, off:off + npix].rearrange("c (p x) -> p c x", p=128),
                )
                # k_t column-block b = k*NV + v holds input channel NK + k*NV + v
                # (k-major matches DRAM channel order → 3D-balanced DMAs);
                # two half-loads so prod can start early
                k4 = k_t.rearrange("p (k v x) -> p k v x", k=NK, v=NV)
                half_ch = NK * NV // 2
                for h in range(2):
                    nc.sync.dma_start(
                        out=k_t.rearrange("p (c x) -> p c x", c=NK * NV)
                        [:, h * half_ch:(h + 1) * half_ch],
                        in_=x[NK + h * half_ch:NK + (h + 1) * half_ch, off:off + npix]
                        .rearrange("c (p x) -> p c x", p=128),
                    )
                nc.scalar.dma_start(
                    out=v_t.rearrange("p (c x) -> p c x", c=NV),
                    in_=x[NK + NK * NV:C, off:off + npix]
                    .rearrange("c (p x) -> p c x", p=128),
                )

                # prod[k,v] = q[k] * K[k,v]   (two broadcast multiplies, one per K half)
                prod = work_pool.tile([128, NK * NV * ncol], mid, name=f"prod{j}", tag="prod")
                p4 = prod.rearrange("p (k v x) -> p k v x", k=NK, v=NV)
                q_b = (
                    q_t.rearrange("p (k x) -> p k x", k=NK)
                    .unsqueeze(2)
                    .broadcast_to((128, NK, NV, ncol))
                )
                nc.vector.tensor_tensor(p4[:, 0:4], q_b[:, 0:4], k4[:, 0:4], mybir.AluOpType.mult)
                nc.vector.tensor_tensor(p4[:, 4:8], q_b[:, 4:8], k4[:, 4:8], mybir.AluOpType.mult)

                # sum over k (outer block index): 3-level pairwise tree (all DVE)
                l1 = work_pool.tile([128, 4 * NV * ncol], mid, name=f"l1_{j}", tag="l1")
                l14 = l1.rearrange("p (k v x) -> p k v x", k=4, v=NV)
                nc.vector.tensor_tensor(l14, p4[:, 0:4], p4[:, 4:8], mybir.AluOpType.add)
                l2 = work_pool.tile([128, 2 * NV * ncol], mid, name=f"l2_{j}", tag="l2")
                l24 = l2.rearrange("p (k v x) -> p k v x", k=2, v=NV)
                nc.vector.tensor_tensor(l24, l14[:, 0:2], l14[:, 2:4], mybir.AluOpType.add)
                qk = pipe_pool.tile([128, NV * ncol], mid, name=f"qk{j}", tag="qk")
                qk4 = qk.rearrange("p (v x) -> p v x", v=NV).unsqueeze(1)
                nc.vector.tensor_tensor(qk4, l24[:, 0:1], l24[:, 1:2], mybir.AluOpType.add)

                # e = exp(qk / sqrt(NK)); softmax denominators over v
                e = pipe_pool.tile([128, NV * ncol], f32, name=f"e{j}", tag="e")
                nc.scalar.activation(e, qk, mybir.ActivationFunctionType.Exp, scale=_SCALE)
                t1 = pipe_pool.tile([128, 4 * ncol], f32, name=f"t1_{j}", tag="t1")
                nc.vector.tensor_tensor(t1, e[:, 0:4 * ncol], e[:, 4 * ncol:], mybir.AluOpType.add)
                t2 = pipe_pool.tile([128, 2 * ncol], f32, name=f"t2_{j}", tag="t2")
                nc.vector.tensor_tensor(t2, t1[:, 0:2 * ncol], t1[:, 2 * ncol:], mybir.AluOpType.add)
                s = pipe_pool.tile([128, ncol], f32, name=f"s{j}", tag="s")
                nc.vector.tensor_tensor(s, t2[:, 0:ncol], t2[:, ncol:], mybir.AluOpType.add)
                r = pipe_pool.tile([128, ncol], f32, name=f"r{j}", tag="r")
                if recip_on_act:
                    # r = exp(-ln s): Log and Exp share one ACT table set
                    ls = pipe_pool.tile([128, ncol], f32, name=f"ls{j}", tag="ls")
                    nc.scalar.activation(ls, s, mybir.ActivationFunctionType.Ln)
                    nc.scalar.activation(r, ls, mybir.ActivationFunctionType.Exp, scale=-1.0)
                else:
                    nc.vector.reciprocal(r, s)

                # out[v] = e[v] * V[v] * r
                m1 = pipe_pool.tile([128, NV * ncol], f32, name=f"m1_{j}", tag="m1")
                nc.vector.tensor_tensor(m1, e, v_t, mybir.AluOpType.mult)
                o = pipe_pool.tile([128, NV * ncol], f32, name=f"o{j}", tag="o")
                r_b = r.unsqueeze(1).broadcast_to((128, NV, ncol))
                nc.vector.tensor_tensor(
                    o.rearrange("p (v x) -> p v x", v=NV),
                    m1.rearrange("p (v x) -> p v x", v=NV),
                    r_b,
                    mybir.AluOpType.mult,
                )
                # one output DMA per chunk on the scalar HWDGE ring
                nc.scalar.dma_start(
                    out=y[0:NV, off:off + npix].rearrange("c (p x) -> p c x", p=128),
                    in_=o.rearrange("p (c x) -> p c x", c=NV),
                )
    nc.compile()
    return nc


_NC_CACHE = {}

# default build configuration used by kernel()
BUILD_CFG = {"recip_on_act": False, "bf16_tree": False}


def _get_nc(**cfg):
    cfg = {**BUILD_CFG, **cfg}
    key = (PIX, NCHUNK, tuple(sorted(cfg.items())))
    if key not in _NC_CACHE:
        _NC_CACHE[key] = build_nc(**cfg)
    return _NC_CACHE[key]


def make_in_maps(inp):
    in_maps = []
    for core in range(N_CORES):
        b, half = core // 2, core % 2
        shard = np.ascontiguousarray(
            inp[b, :, half * ROWS:(half + 1) * ROWS, :], dtype=np.float32
        ).reshape(C, PIX)
        in_maps.append({"x": shard})
    return in_maps


def assemble_out(results):
    out = np.empty((B, NV, H, W), np.float32)
    for core in range(N_CORES):
        b, half = core // 2, core % 2
        out[b, :, half * ROWS:(half + 1) * ROWS, :] = (
            results[core]["y"].reshape(NV, ROWS, W)
        )
    return out


def run_spmd(inp, trace=False, build_cfg=None, **kwargs):
    """Run the SPMD kernel on 8 cores; returns (full_output, BassKernelResults)."""
    _ensure_path()
    from concourse.bass_utils import run_bass_kernel_spmd

    inp = np.asarray(inp)
    assert inp.shape == (B, C, H, W), inp.shape
    nc = _get_nc(**(build_cfg or {}))
    res = run_bass_kernel_spmd(
        nc, make_in_maps(inp), list(range(N_CORES)), trace=trace, **kwargs
    )
    return assemble_out(res.results), res


def kernel(inp):
    out, _ = run_spmd(inp, trace=False)
    return out
